# Initial kernel scaffold
#
"""CANLayer (multi-head GNN attention message passing) on 8 Trainium2 cores.

Strategy (self-contained; shapes hardcoded for the harness problem):
  - Shard by TARGET node: core c owns target rows [c*6250, (c+1)*6250).
    All segment (softmax) reductions become core-local -> zero collectives.
  - Device stage A builds per-edge-set node tables tab[N+2, 256] bf16-rows:
    [xm 4x32 bf16 (256B) | s_src 4 f32 | s_tgt 4 f32 | zero pad] = 512B/row,
    via one [128,128]@[128,192] matmul per 128-node tile (lhsT = host-
    transposed x tile, rhs assembled on device from W and a). Rows 0 and N+1
    are dummies (xm=0, s_src=-1e30, s_tgt=0) used for ELL padding.
  - Host preprocessing builds, per (edge set, source segment) "structure", a
    degree-sorted variable-width ELL layout. Sources are segmented at node
    SPLIT so dma_gather's int16 indices cover each table view. Per structure:
    targets sorted by in-degree desc, tiled by 128 rows, tile t gets K[t]
    slots (max over cores -> SPMD-identical program).
  - Stage B per structure: bulk dma_gather pulls 512B source rows in
    column-chunks; DVE computes exp(leaky_relu(s_src+s_tgt)*val) per head
    (4 heads in one op via 4D APs), scales gathered xm, reduces over slots;
    unnormalized num[128] + den[4] accumulate in SBUF and one dma_scatter_add
    per structure adds them into a node-ordered DRAM buffer [NPC+1, 192].
  - Final pass: out = relu(num/max(den,1e-16) per head + EPS * x_own@W_skip).

Softmax max-subtraction is skipped deliberately: real scores are bounded
(|e| < ~6) so exp never overflows; softmax is shift-invariant. Pad slots
gather a dummy row with s_src=-1e30; scores are clamped to >= -85 before exp
because HW ACT Exp of huge-negative inputs is garbage rather than 0.
"""

import numpy as np
from dataclasses import dataclass

import concourse.bass as bass
import concourse.bacc as bacc
import concourse.mybir as mybir
from concourse.tile import TileContext
from concourse.bass_utils import run_bass_kernel_spmd

F32 = mybir.dt.float32
BF16 = mybir.dt.bfloat16
I16 = mybir.dt.int16
I32 = mybir.dt.int32
AF = mybir.ActivationFunctionType
ALU = mybir.AluOpType

P = 128
IN_CH = 128
OUT_CH = 32
HEADS = 4
RB = 256          # table row width in bf16 elems (512 bytes)
RF = RB // 2      # row width in f32 elems (128)
NR = 192          # num-buffer row width in f32 (128 num + 4 den + pad)
EPS = 1.0 + 1e-06
SLOPE = 0.01
NEG = -1.0e30
CHUNK = 8         # gather chunk width (columns); 8*128 = 1024 idxs,
                  # the max one dma_gather survives on HW
DEBUG_NSTRUCT = None  # limit number of stage-B structures (debug)
DEBUG_PARTS = 99      # structure body parts: 1 blobs, 2 +own, 3 +merge, 4 +chunks, 5 +scatter
DEBUG_TILEOPS = 99    # 0 gather only, 1 +scores/den, 2 +mult/reduce


@dataclass
class Cfg:
    N: int = 50000
    C: int = 8
    SPLIT: int = 32000    # source segment split (int16 range for dma_gather)

    @property
    def NPC(self):
        return self.N // self.C

    @property
    def T(self):
        return -(-self.NPC // P)

    @property
    def RPC(self):
        return self.T * P

    @property
    def NTAB(self):       # rows: 0 dummy | 1..N nodes | N+1 dummy
        return self.N + 2


def _wrap16(seq: np.ndarray) -> np.ndarray:
    """int sequence [n] (n % 16 == 0) -> dma_gather idx layout [128, n/16]."""
    w = seq.reshape(-1, 16).T.astype(np.int16)
    return np.ascontiguousarray(np.tile(w, (8, 1)))


def _prep_structure(cfg: Cfg, i, j, v, seg: int):
    """Build per-core blobs for one (edge set, source segment) structure.

    seg 0: sources < SPLIT (table view A, idx = src+1, pad 0)
    seg 1: sources >= SPLIT (table view B, idx = src-SPLIT, pad N-SPLIT)
    Returns (K, offs, SK, per_core list of dicts).
    """
    NPC, T, RPC, N, S = cfg.NPC, cfg.T, cfg.RPC, cfg.N, cfg.SPLIT
    if seg == 0:
        m_seg = j < S
    else:
        m_seg = j >= S
    i, j, v = i[m_seg], j[m_seg], v[m_seg]
    core = i // NPC

    percore = []
    for c in range(cfg.C):
        m = core == c
        il = (i[m] - c * NPC).astype(np.int64)
        jl = j[m].astype(np.int64)
        vl = v[m]
        deg = np.bincount(il, minlength=NPC)
        order = np.argsort(-deg, kind="stable")
        rank = np.empty(NPC, np.int64)
        rank[order] = np.arange(NPC)
        percore.append((il, jl, vl, deg, order, rank))

    K = np.zeros(T, np.int64)
    for c in range(cfg.C):
        ds = np.zeros(RPC, np.int64)
        ds[:NPC] = percore[c][3][percore[c][4]]
        K = np.maximum(K, ds.reshape(T, P).max(axis=1))
    K = np.maximum(K, 1)
    offs = np.concatenate([[0], np.cumsum(K)]).astype(np.int64)
    SK = int(offs[-1])

    pad_idx = 0 if seg == 0 else N + 1
    out = []

    def locb_i32(loc):
        return loc.astype(np.int32).reshape(T, P).T.copy()
    for c in range(cfg.C):
        il, jl, vl, deg, order, rank = percore[c]
        # gather index grid [SK cols, 128 rows] then flatten column-major
        gidx = np.full((SK, P), pad_idx, np.int64)
        val_blob = np.ones((P, SK), np.float32)
        pos = rank[il]
        o = np.argsort(pos, kind="stable")
        pos_s = pos[o]
        slot = np.arange(len(pos_s)) - np.searchsorted(pos_s, pos_s)
        t = pos_s // P
        p = pos_s % P
        col = offs[t] + slot
        gidx[col, p] = jl[o] + 1
        val_blob[p, col] = vl[o]

        # own target rows (global table row ids), perm order, pads -> 0
        pos_all = np.arange(RPC)
        valid = pos_all < NPC
        grow = np.zeros(RPC, np.int64)
        grow[valid] = order[pos_all[valid]] + c * NPC + 1
        # scatter: local node id or NPC (trash)
        loc = np.full(RPC, NPC, np.int64)
        loc[valid] = order[pos_all[valid]]
        out.append(dict(
            gidx=np.ascontiguousarray(gidx.T.astype(np.int32)),   # [P, SK]
            val=val_blob,
            own=grow.astype(np.int32).reshape(T, P).T.copy(),     # [P, T]
            scat=locb_i32(loc),
        ))
    return K, offs, SK, out


def _chunks(K, offs, width=CHUNK):
    """Group consecutive tiles into gather chunks of <= width columns
    (a single tile wider than `width` gets its own chunk).
    Returns list of (col_start, col_width, [tile indices])."""
    res = []
    cur: list[int] = []
    for t in range(len(K)):
        kt = int(K[t])
        if cur and int(offs[t] + kt - offs[cur[0]]) > width:
            res.append((int(offs[cur[0]]), int(offs[cur[-1] + 1] - offs[cur[0]]), cur))
            cur = []
        cur.append(t)
    res.append((int(offs[cur[0]]), int(offs[cur[-1] + 1] - offs[cur[0]]), cur))
    return res


def _build_nc(cfg: Cfg, meta):
    """meta: dict (set,seg) -> (K, offs, SK)."""
    sets = ("low", "up")
    N, NPC, T, NTAB, S = cfg.N, cfg.NPC, cfg.T, cfg.NTAB, cfg.SPLIT
    RPC = cfg.RPC

    nc = bacc.Bacc("TRN2", target_bir_lowering=False, debug=False)

    xT = nc.dram_tensor("xT", [P, N], F32, kind="ExternalInput")
    xoT = nc.dram_tensor("xoT", [P, NPC], F32, kind="ExternalInput")
    Wsk = nc.dram_tensor("W_skip", [IN_CH, HEADS * OUT_CH], F32, kind="ExternalInput")
    W = {s: nc.dram_tensor(f"W_{s}", [HEADS, IN_CH, OUT_CH], F32, kind="ExternalInput")
         for s in sets}
    A = {s: nc.dram_tensor(f"ab_{s}", [P, 2 * HEADS], F32, kind="ExternalInput")
         for s in sets}

    gidx_d, val_d, ownA_d, ownB_d, scat_d = {}, {}, {}, {}, {}
    for key, (K, offs, SK) in meta.items():
        s, g = key
        gidx_d[key] = nc.dram_tensor(f"gidx_{s}{g}", [P, SK], I32,
                                     kind="ExternalInput")
        val_d[key] = nc.dram_tensor(f"val_{s}{g}", [P, SK], F32,
                                    kind="ExternalInput")
        ownA_d[key] = nc.dram_tensor(f"own_{s}{g}", [P, T], I32,
                                     kind="ExternalInput")
        scat_d[key] = nc.dram_tensor(f"scat_{s}{g}", [P, T], I32,
                                     kind="ExternalInput")

    tab = {s: nc.dram_tensor(f"tab_{s}", [NTAB * RB], BF16) for s in sets}
    numb = {(s, g): nc.dram_tensor(f"numb_{s}{g}", [(NPC + 1) * NR], F32)
            for s in sets for g in range(2)}
    out_d = nc.dram_tensor("out", [NPC, IN_CH], F32, kind="ExternalOutput")

    n_atiles = -(-N // P)

    with TileContext(nc) as tc:
        with (
            tc.tile_pool(name="const", bufs=1) as cpool,
            tc.tile_pool(name="setup_ps", bufs=1, space="PSUM") as spsum,
            tc.tile_pool(name="stagea", bufs=3) as apool,
            tc.tile_pool(name="stagea_ps", bufs=3, space="PSUM") as apsum,
            tc.tile_pool(name="blob", bufs=1) as bpool,
            tc.tile_pool(name="gather", bufs=2) as gpool,
            tc.tile_pool(name="small", bufs=3) as upool,
            tc.tile_pool(name="fin", bufs=3) as fpool,
            tc.tile_pool(name="fin_ps", bufs=2, space="PSUM") as fpsum,
        ):
            from concourse.masks import make_identity

            ident = cpool.tile([P, P], F32)
            make_identity(nc, ident[:])
            wsk_sb = cpool.tile([IN_CH, HEADS * OUT_CH], F32)
            nc.sync.dma_start(out=wsk_sb[:], in_=Wsk[:, :])

            # ---- M matrices: [128 in_ch, 192]: cols 0-127 W heads,
            #      128-131 src score cols, 132-135 tgt score cols, rest 0
            M = {}
            for s in sets:
                m_sb = cpool.tile([IN_CH, NR], F32, tag=f"M_{s}")
                nc.vector.memset(m_sb[:], 0.0)
                nc.sync.dma_start(
                    out=m_sb[:, 0:IN_CH].rearrange("p (h f) -> p h f", h=HEADS),
                    in_=W[s][:, :, :].rearrange("h p f -> p h f"),
                )
                wt_ps = spsum.tile([P, P], F32, tag="wt_ps")
                nc.tensor.transpose(out=wt_ps[:], in_=m_sb[:, 0:IN_CH],
                                    identity=ident[:])
                wt_sb = upool.tile([P, P], F32, tag="wt_sb")
                nc.vector.tensor_copy(out=wt_sb[:], in_=wt_ps[:])
                a_bd = upool.tile([P, 2 * HEADS], F32, tag="a_bd")
                nc.sync.dma_start(out=a_bd[:], in_=A[s][:, :])
                col_ps = spsum.tile([P, 8], F32, tag="col_ps")
                nc.tensor.matmul(out=col_ps[:], lhsT=wt_sb[:], rhs=a_bd[:],
                                 start=True, stop=True)
                nc.vector.tensor_copy(out=m_sb[:, IN_CH:IN_CH + 8], in_=col_ps[:])
                M[s] = m_sb

            # ---- stage A: build tables ----
            tabv = {s: tab[s][:].rearrange("(n e) -> n e", e=RB) for s in sets}
            tabf = {s: tab[s][:].bitcast(F32).rearrange("(n e) -> n e", e=RF)
                    for s in sets}
            for ti in range(n_atiles):
                st = ti * P
                n = min(P, N - st)
                xt = apool.tile([P, n], F32, tag="xt")
                nc.sync.dma_start(out=xt[:], in_=xT[:, st:st + n])
                for s in sets:
                    ps = apsum.tile([P, NR], F32, tag="a_ps")
                    nc.tensor.matmul(out=ps[:n, :], lhsT=xt[:], rhs=M[s][:],
                                     start=True, stop=True)
                    sb = apool.tile([P, RB], BF16, tag=f"a_sb_{s}")
                    nc.scalar.activation(
                        out=sb[:n, 0:IN_CH], in_=ps[:n, 0:IN_CH], func=AF.Copy,
                    )
                    nc.vector.tensor_copy(
                        out=sb[:].bitcast(F32)[:n, RF // 2:RF],
                        in_=ps[:n, IN_CH:IN_CH + 64],
                    )
                    nc.sync.dma_start(out=tabv[s][1 + st:1 + st + n, :],
                                      in_=sb[:n, :])
            # dummy rows 0 and N+1: xm=0, s_src=-1e30, s_tgt=0
            dmy = cpool.tile([1, RF], F32)
            nc.vector.memset(dmy[:], 0.0)
            nc.vector.memset(dmy[:, RF // 2:RF // 2 + 4], NEG)
            for s in sets:
                nc.sync.dma_start(out=tabf[s][0:1, :], in_=dmy[:])
                nc.sync.dma_start(out=tabf[s][N + 1:N + 2, :], in_=dmy[:])

            numv = {k: numb[k][:].rearrange("(n e) -> n e", e=NR)
                    for k in numb}

            # ---- stage B: 4 structures ----
            _nstruct = 0
            for s in sets:
                for g in range(2):
                    _nstruct += 1
                    if DEBUG_NSTRUCT is not None and _nstruct > DEBUG_NSTRUCT:
                        continue
                    key = (s, g)
                    K, offs, SK = meta[key]
                    gi = bpool.tile([P, SK], I32, tag="gi")
                    nc.sync.dma_start(out=gi[:], in_=gidx_d[key][:, :])
                    vb = bpool.tile([P, SK], F32, tag="vb")
                    nc.sync.dma_start(out=vb[:], in_=val_d[key][:, :])
                    oa_i = bpool.tile([P, T], I32, tag="oa")
                    nc.sync.dma_start(out=oa_i[:], in_=ownA_d[key][:, :])
                    sc_i = bpool.tile([P, T], I32, tag="sc")
                    nc.sync.dma_start(out=sc_i[:], in_=scat_d[key][:, :])

                    if DEBUG_PARTS < 2:
                        continue
                    ownA = bpool.tile([P, T * RB], BF16, tag="ownA")
                    ownAv = ownA[:].rearrange("p (c e) -> p c e", e=RB)
                    for t in range(T):
                        nc.gpsimd.indirect_dma_start(
                            out=ownAv[:, t, :], out_offset=None,
                            in_=tabv[s],
                            in_offset=bass.IndirectOffsetOnAxis(
                                ap=oa_i[:, t:t + 1], axis=0),
                        )
                    ownf = ownA[:].bitcast(F32).rearrange(
                        "p (c q) -> p c q", q=RF)

                    na = bpool.tile([P, T * NR], F32, tag="na")
                    nc.vector.memset(na[:], 0.0)
                    nav = na[:].rearrange("p (t q) -> p t q", q=NR)

                    if DEBUG_PARTS < 4:
                        continue
                    for (c0, cw, tiles) in _chunks(K, offs):
                        G = gpool.tile([P, cw * RB], BF16, tag="G")
                        Gg = G[:].rearrange("p (c e) -> p c e", e=RB)
                        for cc in range(cw):
                            nc.gpsimd.indirect_dma_start(
                                out=Gg[:, cc, :], out_offset=None,
                                in_=tabv[s],
                                in_offset=bass.IndirectOffsetOnAxis(
                                    ap=gi[:, c0 + cc:c0 + cc + 1], axis=0),
                            )
                        Gf = G[:].bitcast(F32).rearrange("p (c q) -> p c q", q=RF)
                        # row is 256 bf16 = 8 "head" blocks of 32; xm = blocks 0:4
                        G4 = G[:].rearrange("p (c h f) -> p c h f",
                                            h=RB // OUT_CH, f=OUT_CH)[:, :, 0:HEADS, :]
                        Gr = G[:].rearrange("p (c h f) -> p h f c",
                                            h=RB // OUT_CH, f=OUT_CH)[:, 0:HEADS, :, :]
                        for t in tiles:
                            if DEBUG_TILEOPS < 1:
                                continue
                            Kt = int(K[t])
                            lc = int(offs[t]) - c0
                            u4 = upool.tile([P, Kt * HEADS], F32, tag="u4")
                            u4v = u4[:].rearrange("p (k h) -> p k h", h=HEADS)
                            # (s_src + s_tgt) * val
                            nc.vector.tensor_add(
                                out=u4v,
                                in0=Gf[:, lc:lc + Kt, RF // 2:RF // 2 + 4],
                                in1=ownf[:, t:t + 1, RF // 2 + 4:RF // 2 + 8]
                                .to_broadcast([P, Kt, HEADS]),
                            )
                            nc.vector.tensor_mul(
                                out=u4v, in0=u4v,
                                in1=vb[:, int(offs[t]):int(offs[t]) + Kt]
                                .rearrange("p (k o) -> p k o", o=1)
                                .to_broadcast([P, Kt, HEADS]),
                            )
                            # leaky relu (val >= 0), clamp, exp
                            nc.vector.scalar_tensor_tensor(
                                out=u4[:], in0=u4[:], scalar=SLOPE, in1=u4[:],
                                op0=ALU.mult, op1=ALU.max,
                            )
                            nc.vector.tensor_scalar_max(
                                out=u4[:], in0=u4[:], scalar1=-85.0)
                            nc.scalar.activation(out=u4[:], in_=u4[:], func=AF.Exp)
                            # den per head
                            nc.vector.tensor_reduce(
                                out=nav[:, t:t + 1, IN_CH:IN_CH + HEADS],
                                in_=u4[:].rearrange("p (k h) -> p h k", h=HEADS),
                                axis=mybir.AxisListType.X, op=ALU.add,
                            )
                            if DEBUG_TILEOPS < 2:
                                continue
                            # scale gathered xm by p, reduce over slots
                            nc.vector.tensor_mul(
                                out=G4[:, lc:lc + Kt, :, :],
                                in0=G4[:, lc:lc + Kt, :, :],
                                in1=u4[:].rearrange(
                                    "p (k h o) -> p k h o", h=HEADS, o=1)
                                .to_broadcast([P, Kt, HEADS, OUT_CH]),
                            )
                            nc.vector.tensor_reduce(
                                out=nav[:, t:t + 1, 0:IN_CH],
                                in_=Gr[:, :, :, lc:lc + Kt],
                                axis=mybir.AxisListType.X, op=ALU.add,
                            )
                    if DEBUG_PARTS < 5:
                        continue
                    for t in range(T):
                        nc.gpsimd.indirect_dma_start(
                            out=numv[key],
                            out_offset=bass.IndirectOffsetOnAxis(
                                ap=sc_i[:, t:t + 1], axis=0),
                            in_=na[:, t * NR:(t + 1) * NR],
                            in_offset=None,
                        )

            # ---- final pass ----
            n_ftiles = -(-NPC // P)
            for t in range(n_ftiles):
                st = t * P
                n = min(P, NPC - st)
                xo = fpool.tile([P, n], F32, tag="xo")
                nc.sync.dma_start(out=xo[:], in_=xoT[:, st:st + n])
                ps = fpsum.tile([P, IN_CH], F32, tag="f_ps")
                nc.tensor.matmul(out=ps[:n, :], lhsT=xo[:], rhs=wsk_sb[:],
                                 start=True, stop=True)
                # each edge set normalizes by its OWN denominator
                accs = []
                for s in sets:
                    nb = fpool.tile([P, NR], F32, tag=f"nb_{s}")
                    nc.sync.dma_start(out=nb[:n, :], in_=numv[(s, 0)][st:st + n, :])
                    nb1 = fpool.tile([P, NR], F32, tag=f"nb1_{s}")
                    nc.sync.dma_start(out=nb1[:n, :],
                                      in_=numv[(s, 1)][st:st + n, :])
                    nc.vector.tensor_add(out=nb[:n, :], in0=nb[:n, :],
                                         in1=nb1[:n, :])
                    rec = upool.tile([P, HEADS], F32, tag=f"rec_{s}")
                    nc.vector.tensor_scalar_max(
                        out=rec[:n], in0=nb[:n, IN_CH:IN_CH + HEADS],
                        scalar1=1e-16)
                    nc.vector.reciprocal(out=rec[:n], in_=rec[:n])
                    acc = fpool.tile([P, IN_CH], F32, tag=f"acc_{s}")
                    nc.vector.tensor_mul(
                        out=acc[:n].rearrange("p (h f) -> p h f", h=HEADS),
                        in0=nb[:n, 0:IN_CH].rearrange("p (h f) -> p h f", h=HEADS),
                        in1=rec[:n].rearrange("p (h o) -> p h o", o=1)
                        .to_broadcast([n, HEADS, OUT_CH]),
                    )
                    accs.append(acc)
                acc = accs[0]
                nc.vector.tensor_add(out=acc[:n, :], in0=acc[:n, :],
                                     in1=accs[1][:n, :])
                # acc += EPS * skip; relu
                nc.vector.scalar_tensor_tensor(
                    out=acc[:n, :], in0=ps[:n, :], scalar=EPS, in1=acc[:n, :],
                    op0=ALU.mult, op1=ALU.add,
                )
                nc.scalar.activation(out=acc[:n, :], in_=acc[:n, :], func=AF.Relu)
                nc.sync.dma_start(out=out_d[st:st + n, :], in_=acc[:n, :])

    nc.compile()
    return nc


def _prep_all(cfg: Cfg, inputs):
    x = np.asarray(inputs["x"], dtype=np.float32)
    xTf = np.ascontiguousarray(x.T)

    meta, blobs = {}, {}
    for s, ikey, vkey in (
        ("low", "lower_indices", "lower_values"),
        ("up", "upper_indices", "upper_values"),
    ):
        idx = np.asarray(inputs[ikey])
        i = np.asarray(idx[0], dtype=np.int64)
        j = np.asarray(idx[1], dtype=np.int64)
        v = np.asarray(inputs[vkey], dtype=np.float32)
        for g in range(2):
            K, offs, SK, pc = _prep_structure(cfg, i, j, v, g)
            meta[(s, g)] = (K, offs, SK)
            blobs[(s, g)] = pc

    nc = _build_nc(cfg, meta)

    def ablob(a):
        a = np.asarray(a, dtype=np.float32)
        ab = np.zeros((P, 2 * HEADS), np.float32)
        for h in range(HEADS):
            ab[h * OUT_CH:(h + 1) * OUT_CH, h] = a[h, :OUT_CH]
            ab[h * OUT_CH:(h + 1) * OUT_CH, HEADS + h] = a[h, OUT_CH:]
        return ab

    in_maps = []
    for c in range(cfg.C):
        m = {
            "xT": xTf,
            "xoT": np.ascontiguousarray(xTf[:, c * cfg.NPC:(c + 1) * cfg.NPC]),
            "W_skip": np.asarray(inputs["W_skip"], dtype=np.float32),
            "W_low": np.asarray(inputs["W_low"], dtype=np.float32),
            "ab_low": ablob(inputs["a_low"]),
            "W_up": np.asarray(inputs["W_up"], dtype=np.float32),
            "ab_up": ablob(inputs["a_up"]),
        }
        for key, pc in blobs.items():
            s, g = key
            b = pc[c]
            m[f"gidx_{s}{g}"] = b["gidx"]
            m[f"val_{s}{g}"] = b["val"]
            m[f"own_{s}{g}"] = b["own"]
            m[f"scat_{s}{g}"] = b["scat"]
        in_maps.append(m)
    return nc, in_maps


def _execute(inputs, trace=False, trace_cores=None, cfg=None):
    if cfg is None:
        cfg = Cfg(N=int(np.asarray(inputs["x"]).shape[0]), C=8)
    nc, in_maps = _prep_all(cfg, inputs)
    res = run_bass_kernel_spmd(
        nc, in_maps, core_ids=list(range(cfg.C)),
        trace=trace, trace_cores=trace_cores,
    )
    out = np.concatenate([res.results[c]["out"] for c in range(cfg.C)], axis=0)
    return out.astype(np.float32), res


def kernel(**inputs) -> np.ndarray:
    return _execute(inputs)[0]



# revision 10
# speedup vs baseline: 1.7517x; 1.7517x over previous
"""CANLayer (multi-head GNN attention message passing) on 8 Trainium2 cores.

Strategy (self-contained; shapes hardcoded for the harness problem):
  - Shard by TARGET node: core c owns target rows [c*6250, (c+1)*6250).
    All segment (softmax) reductions become core-local -> zero collectives.
  - Device stage A builds per-edge-set node tables tab[N+2] of 512B rows:
    [xm 4x32 bf16 (256B) | s_src 4 f32 | zero pad], via one [128,128]@[128,136]
    matmul per 128-node tile. Rows 0 and N+1 are dummies (xm=0, s_src=-7000)
    used for ELL padding; -7000 survives leaky-relu (x0.01) as -70, whose exp
    underflows to ~0 without needing a clamp op.
  - Host preprocessing builds, per (edge set, source segment) "structure", a
    degree-sorted variable-width ELL layout. Sources are segmented at node
    SPLIT=32000 so dma_gather's int16 indices cover each table view. Per
    structure: targets sorted by in-degree desc, tiled by 128 rows, tile t
    gets K[t] slots (max over cores -> SPMD-identical program).
  - Stage B per structure: bulk dma_gather (1024 idx / call -- ~1us SWDGE
    fixed cost amortized over 8 columns) pulls 512B source rows; s_tgt comes
    from a tiny per-structure matmul over host-permuted x columns (xoP blob)
    instead of gathering own rows. DVE computes exp(lrelu(s_src+s_tgt)*val)
    per head, scales gathered xm, reduces over slots via a bf16 pairwise
    tree (2x DVE mode) for large tiles / tensor_reduce for small ones.
    Unnormalized num[128]+den[4] rows scatter via batched dma_scatter_add
    into a zero-initialized node-ordered DRAM buffer [NPC+2, 192].
  - Final pass: out = relu(num/max(den,1e-16) per head + EPS * x_own@W_skip).

Softmax max-subtraction is skipped deliberately: real scores are bounded
(|e| < ~6) so exp never overflows; softmax is shift-invariant.
"""

import numpy as np
from dataclasses import dataclass

import concourse.bass as bass
import concourse.bacc as bacc
import concourse.mybir as mybir
from concourse.tile import TileContext
from concourse.bass_utils import run_bass_kernel_spmd

F32 = mybir.dt.float32
BF16 = mybir.dt.bfloat16
I16 = mybir.dt.int16
AF = mybir.ActivationFunctionType
ALU = mybir.AluOpType

P = 128
IN_CH = 128
OUT_CH = 32
HEADS = 4
RB = 256          # table row width in bf16 elems (512 bytes)
RF = RB // 2      # row width in f32 elems (128)
MW = IN_CH + 8    # stage-A matmul width: 128 xm + 4 s_src + 4 s_tgt
NR = 192          # num-buffer row width in f32 (128 num + 4 den + pad)
EPS = 1.0 + 1e-06
SLOPE = 0.01
NEG = -7000.0     # pad s_src; lrelu -> -70, exp -> 4e-31 (HW-safe, no clamp)
GCALL = 4         # dma_gather columns per call (4*128 = 512 idxs; 1024 is
                  # the HW max but runs at the edge -- 512 adds margin)
GC = 32           # gather super-chunk width in columns (SBUF G tile)
TREE_MIN = 12     # use bf16 tree-reduce for Kt >= this, tensor_reduce below


@dataclass
class Cfg:
    N: int = 50000
    C: int = 8
    SPLIT: int = 32000    # source segment split (int16 range for dma_gather)

    @property
    def NPC(self):
        return self.N // self.C

    @property
    def T(self):
        return -(-self.NPC // P)

    @property
    def RPC(self):
        return self.T * P

    @property
    def NTAB(self):       # rows: 0 dummy | 1..N nodes | N+1 dummy
        return self.N + 2

    @property
    def NBROWS(self):     # num-buffer rows: NPC real + trash + spare
        return self.NPC + 2


def _wrap16(seq: np.ndarray) -> np.ndarray:
    """int sequence [n] (n % 16 == 0) -> dma_gather idx layout [128, n/16]."""
    w = seq.reshape(-1, 16).T.astype(np.int16)
    return np.ascontiguousarray(np.tile(w, (8, 1)))


def _prep_structure(cfg: Cfg, i, j, v, seg: int):
    """Build per-core blobs for one (edge set, source segment) structure.

    seg 0: sources < SPLIT (table view rows [0, S+1), idx = src+1, pad 0)
    seg 1: sources >= SPLIT (view rows [S+1, N+2), idx = src-S, pad N-S)
    Returns (K, offs, SK, per_core list of dicts).
    """
    NPC, T, RPC, N, S = cfg.NPC, cfg.T, cfg.RPC, cfg.N, cfg.SPLIT
    if seg == 0:
        m_seg = j < S
    else:
        m_seg = j >= S
    i, j, v = i[m_seg], j[m_seg], v[m_seg]
    core = i // NPC

    percore = []
    for c in range(cfg.C):
        m = core == c
        il = (i[m] - c * NPC).astype(np.int64)
        jl = j[m].astype(np.int64)
        vl = v[m]
        deg = np.bincount(il, minlength=NPC)
        order = np.argsort(-deg, kind="stable")
        rank = np.empty(NPC, np.int64)
        rank[order] = np.arange(NPC)
        percore.append((il, jl, vl, deg, order, rank))

    K = np.zeros(T, np.int64)
    for c in range(cfg.C):
        ds = np.zeros(RPC, np.int64)
        ds[:NPC] = percore[c][3][percore[c][4]]
        K = np.maximum(K, ds.reshape(T, P).max(axis=1))
    K = np.maximum(K, 1)
    offs = np.concatenate([[0], np.cumsum(K)]).astype(np.int64)
    SK = int(offs[-1])

    # view-local pad index (dummy table row)
    pad_idx = 0 if seg == 0 else N - S
    out = []
    for c in range(cfg.C):
        il, jl, vl, deg, order, rank = percore[c]
        # gather index grid [SK cols, 128 rows]; flat order = col*128 + p
        gidx = np.full((SK, P), pad_idx, np.int64)
        val_blob = np.ones((P, SK), np.float32)
        pos = rank[il]
        o = np.argsort(pos, kind="stable")
        pos_s = pos[o]
        slot = np.arange(len(pos_s)) - np.searchsorted(pos_s, pos_s)
        t = pos_s // P
        p = pos_s % P
        col = offs[t] + slot
        # view-local source index
        jloc = jl[o] + 1 if seg == 0 else jl[o] - S
        gidx[col, p] = jloc
        val_blob[p, col] = vl[o]

        # scatter: local node id per (tile, partition); pads -> trash row NPC
        pos_all = np.arange(RPC)
        valid = pos_all < NPC
        loc = np.full(RPC, NPC, np.int64)
        loc[valid] = order[pos_all[valid]]

        out.append(dict(
            gidx16=_wrap16(gidx.reshape(-1)),          # [128, SK*8]
            val=val_blob,                              # [P, SK]
            scat16=_wrap16(loc),                       # [128, T*8]
            perm=order,                                # host: for xoP build
        ))
    return K, offs, SK, out


def _chunks(K, offs, width=GC):
    """Group consecutive tiles into gather chunks of <= width columns
    (a single tile wider than `width` gets its own chunk).
    Returns list of (col_start, col_width, [tile indices])."""
    res = []
    cur: list[int] = []
    for t in range(len(K)):
        kt = int(K[t])
        if cur and int(offs[t] + kt - offs[cur[0]]) > width:
            res.append((int(offs[cur[0]]), int(offs[cur[-1] + 1] - offs[cur[0]]), cur))
            cur = []
        cur.append(t)
    res.append((int(offs[cur[0]]), int(offs[cur[-1] + 1] - offs[cur[0]]), cur))
    return res


def _build_nc(cfg: Cfg, meta):
    """meta: dict (set,seg) -> (K, offs, SK)."""
    sets = ("low", "up")
    N, NPC, T, NTAB, S = cfg.N, cfg.NPC, cfg.T, cfg.NTAB, cfg.SPLIT
    RPC, NBR = cfg.RPC, cfg.NBROWS
    NBF = NBR * NR          # flat f32 size of one num buffer
    assert NBF % P == 0
    NBW = NBF // P          # zero-fill row width


    nc = bacc.Bacc("TRN2", target_bir_lowering=False, debug=False,
                   num_swdge_queues=2)

    xT = nc.dram_tensor("xT", [P, N], F32, kind="ExternalInput")
    xoT = nc.dram_tensor("xoT", [P, NPC], F32, kind="ExternalInput")
    Wsk = nc.dram_tensor("W_skip", [IN_CH, HEADS * OUT_CH], F32, kind="ExternalInput")
    W = {s: nc.dram_tensor(f"W_{s}", [HEADS, IN_CH, OUT_CH], F32, kind="ExternalInput")
         for s in sets}
    A = {s: nc.dram_tensor(f"ab_{s}", [P, 2 * HEADS], F32, kind="ExternalInput")
         for s in sets}

    gidx_d, val_d, scat_d, xop_d = {}, {}, {}, {}
    for key, (K, offs, SK) in meta.items():
        s, g = key
        gidx_d[key] = nc.dram_tensor(f"gidx_{s}{g}", [P, SK * 8], I16,
                                     kind="ExternalInput")
        val_d[key] = nc.dram_tensor(f"val_{s}{g}", [P, SK], F32,
                                    kind="ExternalInput")
        scat_d[key] = nc.dram_tensor(f"scat_{s}{g}", [P, T * 8], I16,
                                     kind="ExternalInput")
        xop_d[key] = nc.dram_tensor(f"xop_{s}{g}", [P, RPC], F32,
                                    kind="ExternalInput")

    tab = {s: nc.dram_tensor(f"tab_{s}", [NTAB * RB], BF16) for s in sets}
    numb = {(s, g): nc.dram_tensor(f"numb_{s}{g}", [NBF], F32)
            for s in sets for g in range(2)}
    out_d = nc.dram_tensor("out", [NPC, IN_CH], F32, kind="ExternalOutput")

    n_atiles = -(-N // P)

    with TileContext(nc) as tc:
        with (
            tc.tile_pool(name="const", bufs=1) as cpool,
            tc.tile_pool(name="setup_ps", bufs=1, space="PSUM") as spsum,
            tc.tile_pool(name="stagea", bufs=3) as apool,
            tc.tile_pool(name="stagea_ps", bufs=3, space="PSUM") as apsum,
            tc.tile_pool(name="blob", bufs=1) as bpool,
            tc.tile_pool(name="a2_ps", bufs=1, space="PSUM") as a2psum,
            tc.tile_pool(name="gather", bufs=2) as gpool,
            tc.tile_pool(name="small", bufs=3) as upool,
            tc.tile_pool(name="fin", bufs=3) as fpool,
            tc.tile_pool(name="fin_ps", bufs=2, space="PSUM") as fpsum,
        ):
            from concourse.masks import make_identity

            ident = cpool.tile([P, P], F32)
            make_identity(nc, ident[:])
            wsk_sb = cpool.tile([IN_CH, HEADS * OUT_CH], F32)
            nc.sync.dma_start(out=wsk_sb[:], in_=Wsk[:, :])

            # ---- zero the num buffers (overlaps with stage A) ----
            # written through the same [rows, NR] view the scatter-adds use,
            # so the WAR/RAW ordering is tracked on identical APs
            zt = cpool.tile([P, NR], F32)
            nc.vector.memset(zt[:], 0.0)
            for key in numb:
                nv = numb[key][:].rearrange("(n e) -> n e", e=NR)
                for q in range(0, NBR, P):
                    n = min(P, NBR - q)
                    nc.sync.dma_start(out=nv[q:q + n, :], in_=zt[:n, :])

            # ---- M matrices: [128 in_ch, 136]: cols 0-127 W heads,
            #      128-131 src score cols, 132-135 tgt score cols
            M = {}
            for s in sets:
                m_sb = cpool.tile([IN_CH, MW], F32, tag=f"M_{s}")
                nc.vector.memset(m_sb[:], 0.0)
                nc.sync.dma_start(
                    out=m_sb[:, 0:IN_CH].rearrange("p (h f) -> p h f", h=HEADS),
                    in_=W[s][:, :, :].rearrange("h p f -> p h f"),
                )
                wt_ps = spsum.tile([P, P], F32, tag="wt_ps")
                nc.tensor.transpose(out=wt_ps[:], in_=m_sb[:, 0:IN_CH],
                                    identity=ident[:])
                wt_sb = upool.tile([P, P], F32, tag="wt_sb")
                nc.vector.tensor_copy(out=wt_sb[:], in_=wt_ps[:])
                a_bd = upool.tile([P, 2 * HEADS], F32, tag="a_bd")
                nc.sync.dma_start(out=a_bd[:], in_=A[s][:, :])
                col_ps = spsum.tile([P, 8], F32, tag="col_ps")
                nc.tensor.matmul(out=col_ps[:], lhsT=wt_sb[:], rhs=a_bd[:],
                                 start=True, stop=True)
                nc.vector.tensor_copy(out=m_sb[:, IN_CH:IN_CH + 8], in_=col_ps[:])
                M[s] = m_sb

            # ---- stage A: build tables ----
            tabv = {s: tab[s][:].rearrange("(n e) -> n e", e=RB) for s in sets}
            tabf = {s: tab[s][:].bitcast(F32).rearrange("(n e) -> n e", e=RF)
                    for s in sets}
            for ti in range(n_atiles):
                st = ti * P
                n = min(P, N - st)
                xt = apool.tile([P, n], F32, tag="xt")
                nc.sync.dma_start(out=xt[:], in_=xT[:, st:st + n])
                for s in sets:
                    ps = apsum.tile([P, MW], F32, tag="a_ps")
                    nc.tensor.matmul(out=ps[:n, :], lhsT=xt[:], rhs=M[s][:],
                                     start=True, stop=True)
                    sb = apool.tile([P, RB], BF16, tag=f"a_sb_{s}")
                    nc.scalar.activation(
                        out=sb[:n, 0:IN_CH], in_=ps[:n, 0:IN_CH], func=AF.Copy,
                    )
                    nc.vector.tensor_copy(
                        out=sb[:].bitcast(F32)[:n, RF // 2:RF // 2 + 4],
                        in_=ps[:n, IN_CH:IN_CH + 4],
                    )
                    nc.sync.dma_start(out=tabv[s][1 + st:1 + st + n, :],
                                      in_=sb[:n, :])
            # dummy rows 0 and N+1: xm=0, s_src=NEG
            dmy = cpool.tile([1, RF], F32)
            nc.vector.memset(dmy[:], 0.0)
            nc.vector.memset(dmy[:, RF // 2:RF // 2 + 4], NEG)
            for s in sets:
                nc.sync.dma_start(out=tabf[s][0:1, :], in_=dmy[:])
                nc.sync.dma_start(out=tabf[s][N + 1:N + 2, :], in_=dmy[:])

            numv = {k: numb[k][:].rearrange("(n e) -> n e", e=NR)
                    for k in numb}

            # ---- stage B: 4 structures ----
            _gq = [0]     # gather queue toggle
            for s in sets:
                for g in range(2):
                    key = (s, g)
                    K, offs, SK = meta[key]
                    gi16 = bpool.tile([P, SK * 8], I16, tag="gi16")
                    nc.sync.dma_start(out=gi16[:], in_=gidx_d[key][:, :])
                    vb = bpool.tile([P, SK], F32, tag="vb")
                    nc.sync.dma_start(out=vb[:], in_=val_d[key][:, :])
                    sc16 = bpool.tile([P, T * 8], I16, tag="sc16")
                    nc.sync.dma_start(out=sc16[:], in_=scat_d[key][:, :])
                    xoP = bpool.tile([P, RPC], F32, tag="xoP")
                    nc.sync.dma_start(out=xoP[:], in_=xop_d[key][:, :])

                    # stage A2: s_tgt for own (permuted) targets
                    stps = a2psum.tile([P, T * HEADS], F32, tag="stps")
                    for t in range(T):
                        nc.tensor.matmul(
                            out=stps[:, t * HEADS:(t + 1) * HEADS],
                            lhsT=xoP[:, t * P:(t + 1) * P],
                            rhs=M[s][:, IN_CH + 4:IN_CH + 8],
                            start=True, stop=True)
                    stP = bpool.tile([P, T * HEADS], F32, tag="stP")
                    nc.vector.tensor_copy(out=stP[:], in_=stps[:])
                    stPv = stP[:].rearrange("p (t h) -> p t h", h=HEADS)

                    na = bpool.tile([P, T * NR], F32, tag="na")
                    nav = na[:].rearrange("p (t q) -> p t q", q=NR)

                    # table view for this segment (int16 index coverage)
                    view = (tabv[s][0:S + 1, :] if g == 0
                            else tabv[s][S + 1:N + 2, :])

                    for (c0, cw, tiles) in _chunks(K, offs):
                        G = gpool.tile([P, cw * RB], BF16, tag="G")
                        Gg = G[:].rearrange("p (c e) -> p c e", e=RB)
                        for q0 in range(0, cw, GCALL):
                            k = min(GCALL, cw - q0)
                            # alternate SWDGE queues: each queue's desc-gen
                            # runs on its own Q7 core pair
                            nc.gpsimd.dma_gather(
                                out_ap=Gg[:, q0:q0 + k, :],
                                in_ap=view,
                                idxs_ap=gi16[:, (c0 + q0) * 8:(c0 + q0 + k) * 8],
                                num_idxs=k * P,
                                num_idxs_reg=k * P,
                                elem_size=RB,
                                queue_num=_gq[0],
                            )
                            _gq[0] ^= 1
                        Gf = G[:].bitcast(F32).rearrange("p (c q) -> p c q", q=RF)
                        G4 = G[:].rearrange("p (c h f) -> p c h f",
                                            h=RB // OUT_CH, f=OUT_CH)[:, :, 0:HEADS, :]
                        Gr = G[:].rearrange("p (c h f) -> p h f c",
                                            h=RB // OUT_CH, f=OUT_CH)[:, 0:HEADS, :, :]
                        Gxm = Gg[:, :, 0:IN_CH]
                        for t in tiles:
                            Kt = int(K[t])
                            lc = int(offs[t]) - c0
                            u4 = upool.tile([P, Kt * HEADS], F32, tag="u4")
                            u4v = u4[:].rearrange("p (k h) -> p k h", h=HEADS)
                            # (s_src + s_tgt) * val
                            nc.vector.tensor_add(
                                out=u4v,
                                in0=Gf[:, lc:lc + Kt, RF // 2:RF // 2 + 4],
                                in1=stPv[:, t:t + 1, :]
                                .to_broadcast([P, Kt, HEADS]),
                            )
                            nc.vector.tensor_mul(
                                out=u4v, in0=u4v,
                                in1=vb[:, int(offs[t]):int(offs[t]) + Kt]
                                .rearrange("p (k o) -> p k o", o=1)
                                .to_broadcast([P, Kt, HEADS]),
                            )
                            # leaky relu (val >= 0); pads underflow exp
                            nc.vector.scalar_tensor_tensor(
                                out=u4[:], in0=u4[:], scalar=SLOPE, in1=u4[:],
                                op0=ALU.mult, op1=ALU.max,
                            )
                            nc.scalar.activation(out=u4[:], in_=u4[:], func=AF.Exp)
                            # den per head
                            nc.vector.tensor_reduce(
                                out=nav[:, t:t + 1, IN_CH:IN_CH + HEADS],
                                in_=u4[:].rearrange("p (k h) -> p h k", h=HEADS),
                                axis=mybir.AxisListType.X, op=ALU.add,
                            )
                            # scale gathered xm by exp-score
                            nc.vector.tensor_mul(
                                out=G4[:, lc:lc + Kt, :, :],
                                in0=G4[:, lc:lc + Kt, :, :],
                                in1=u4[:].rearrange(
                                    "p (k h o) -> p k h o", h=HEADS, o=1)
                                .to_broadcast([P, Kt, HEADS, OUT_CH]),
                            )
                            # reduce over slots -> num
                            if Kt >= TREE_MIN:
                                m_, base = Kt, lc
                                while m_ > 1:
                                    h_ = m_ // 2
                                    so = m_ - h_
                                    nc.vector.tensor_add(
                                        out=Gxm[:, base:base + h_, :],
                                        in0=Gxm[:, base:base + h_, :],
                                        in1=Gxm[:, base + so:base + m_, :],
                                    )
                                    m_ = so
                                nc.vector.tensor_copy(
                                    out=nav[:, t:t + 1, 0:IN_CH],
                                    in_=Gxm[:, base:base + 1, :],
                                )
                            else:
                                nc.vector.tensor_reduce(
                                    out=nav[:, t:t + 1, 0:IN_CH],
                                    in_=Gr[:, :, :, lc:lc + Kt],
                                    axis=mybir.AxisListType.X, op=ALU.add,
                                )
                    # batched scatter-add into the node-ordered num buffer
                    for q0 in range(0, T, GCALL):
                        k = min(GCALL, T - q0)
                        nc.gpsimd.dma_scatter_add(
                            out_ap=numv[key],
                            in_ap=na[:, q0 * NR:(q0 + k) * NR]
                            .rearrange("p (c e) -> p c e", e=NR),
                            idxs_ap=sc16[:, q0 * 8:(q0 + k) * 8],
                            num_idxs=k * P,
                            num_idxs_reg=k * P,
                            elem_size=NR,
                        )

            # ---- final pass ----
            n_ftiles = -(-NPC // P)
            for t in range(n_ftiles):
                st = t * P
                n = min(P, NPC - st)
                xo = fpool.tile([P, n], F32, tag="xo")
                nc.sync.dma_start(out=xo[:], in_=xoT[:, st:st + n])
                ps = fpsum.tile([P, IN_CH], F32, tag="f_ps")
                nc.tensor.matmul(out=ps[:n, :], lhsT=xo[:], rhs=wsk_sb[:],
                                 start=True, stop=True)
                # each edge set normalizes by its OWN denominator
                accs = []
                for s in sets:
                    nb = fpool.tile([P, NR], F32, tag=f"nb_{s}")
                    nc.sync.dma_start(out=nb[:n, :], in_=numv[(s, 0)][st:st + n, :])
                    nb1 = fpool.tile([P, NR], F32, tag=f"nb1_{s}")
                    nc.sync.dma_start(out=nb1[:n, :],
                                      in_=numv[(s, 1)][st:st + n, :])
                    nc.vector.tensor_add(out=nb[:n, :], in0=nb[:n, :],
                                         in1=nb1[:n, :])
                    rec = upool.tile([P, HEADS], F32, tag=f"rec_{s}")
                    nc.vector.tensor_scalar_max(
                        out=rec[:n], in0=nb[:n, IN_CH:IN_CH + HEADS],
                        scalar1=1e-16)
                    nc.vector.reciprocal(out=rec[:n], in_=rec[:n])
                    acc = fpool.tile([P, IN_CH], F32, tag=f"acc_{s}")
                    nc.vector.tensor_mul(
                        out=acc[:n].rearrange("p (h f) -> p h f", h=HEADS),
                        in0=nb[:n, 0:IN_CH].rearrange("p (h f) -> p h f", h=HEADS),
                        in1=rec[:n].rearrange("p (h o) -> p h o", o=1)
                        .to_broadcast([n, HEADS, OUT_CH]),
                    )
                    accs.append(acc)
                acc = accs[0]
                nc.vector.tensor_add(out=acc[:n, :], in0=acc[:n, :],
                                     in1=accs[1][:n, :])
                # acc += EPS * skip; relu
                nc.vector.scalar_tensor_tensor(
                    out=acc[:n, :], in0=ps[:n, :], scalar=EPS, in1=acc[:n, :],
                    op0=ALU.mult, op1=ALU.add,
                )
                nc.scalar.activation(out=acc[:n, :], in_=acc[:n, :], func=AF.Relu)
                nc.sync.dma_start(out=out_d[st:st + n, :], in_=acc[:n, :])

    nc.compile()
    return nc


def _prep_all(cfg: Cfg, inputs):
    x = np.asarray(inputs["x"], dtype=np.float32)
    xTf = np.ascontiguousarray(x.T)

    meta, blobs = {}, {}
    for s, ikey, vkey in (
        ("low", "lower_indices", "lower_values"),
        ("up", "upper_indices", "upper_values"),
    ):
        idx = np.asarray(inputs[ikey])
        i = np.asarray(idx[0], dtype=np.int64)
        j = np.asarray(idx[1], dtype=np.int64)
        v = np.asarray(inputs[vkey], dtype=np.float32)
        for g in range(2):
            K, offs, SK, pc = _prep_structure(cfg, i, j, v, g)
            meta[(s, g)] = (K, offs, SK)
            blobs[(s, g)] = pc

    nc = _build_nc(cfg, meta)

    def ablob(a):
        a = np.asarray(a, dtype=np.float32)
        ab = np.zeros((P, 2 * HEADS), np.float32)
        for h in range(HEADS):
            ab[h * OUT_CH:(h + 1) * OUT_CH, h] = a[h, :OUT_CH]
            ab[h * OUT_CH:(h + 1) * OUT_CH, HEADS + h] = a[h, OUT_CH:]
        return ab

    NPC, RPC = cfg.NPC, cfg.RPC
    in_maps = []
    for c in range(cfg.C):
        m = {
            "xT": xTf,
            "xoT": np.ascontiguousarray(xTf[:, c * NPC:(c + 1) * NPC]),
            "W_skip": np.asarray(inputs["W_skip"], dtype=np.float32),
            "W_low": np.asarray(inputs["W_low"], dtype=np.float32),
            "ab_low": ablob(inputs["a_low"]),
            "W_up": np.asarray(inputs["W_up"], dtype=np.float32),
            "ab_up": ablob(inputs["a_up"]),
        }
        for key, pc in blobs.items():
            s, g = key
            b = pc[c]
            xop = np.zeros((P, RPC), np.float32)
            xop[:, :NPC] = xTf[:, b["perm"] + c * NPC]
            m[f"gidx_{s}{g}"] = b["gidx16"]
            m[f"val_{s}{g}"] = b["val"]
            m[f"scat_{s}{g}"] = b["scat16"]
            m[f"xop_{s}{g}"] = xop
        in_maps.append(m)
    return nc, in_maps


def _execute(inputs, trace=False, trace_cores=None, cfg=None):
    if cfg is None:
        cfg = Cfg(N=int(np.asarray(inputs["x"]).shape[0]), C=8)
    nc, in_maps = _prep_all(cfg, inputs)
    res = run_bass_kernel_spmd(
        nc, in_maps, core_ids=list(range(cfg.C)),
        trace=trace, trace_cores=trace_cores,
    )
    out = np.concatenate([res.results[c]["out"] for c in range(cfg.C)], axis=0)
    return out.astype(np.float32), res


def kernel(**inputs) -> np.ndarray:
    return _execute(inputs)[0]


# revision 11
# speedup vs baseline: 2.0590x; 1.1754x over previous
"""CANLayer (multi-head GNN attention message passing) on 8 Trainium2 cores.

Strategy (self-contained; shapes hardcoded for the harness problem):
  - Shard by TARGET node: core c owns target rows [c*6250, (c+1)*6250).
    All segment (softmax) reductions become core-local -> zero collectives.
  - Device stage A builds per-edge-set node tables tab[N+2] of 512B rows:
    [xm 4x32 bf16 (256B) | s_src 4 f32 | zero pad], via one [128,128]@[128,136]
    matmul per 128-node tile. Rows 0 and N+1 are dummies (xm=0, s_src=-7000)
    used for ELL padding; -7000 survives leaky-relu (x0.01) as -70, whose exp
    underflows to ~0 without needing a clamp op.
  - Host preprocessing builds, per (edge set, source segment) "structure", a
    degree-sorted variable-width ELL layout. Sources are segmented at node
    SPLIT=32000 so dma_gather's int16 indices cover each table view. Per
    structure: targets sorted by in-degree desc, tiled by 128 rows, tile t
    gets K[t] slots (max over cores -> SPMD-identical program).
  - Stage B per structure: bulk dma_gather (1024 idx / call -- ~1us SWDGE
    fixed cost amortized over 8 columns) pulls 512B source rows; s_tgt comes
    from a tiny per-structure matmul over host-permuted x columns (xoP blob)
    instead of gathering own rows. DVE computes exp(lrelu(s_src+s_tgt)*val)
    per head, scales gathered xm, reduces over slots via a bf16 pairwise
    tree (2x DVE mode) for large tiles / tensor_reduce for small ones.
    Unnormalized num[128]+den[4] rows scatter via batched dma_scatter_add
    into a zero-initialized node-ordered DRAM buffer [NPC+2, 192].
  - Final pass: out = relu(num/max(den,1e-16) per head + EPS * x_own@W_skip).

Softmax max-subtraction is skipped deliberately: real scores are bounded
(|e| < ~6) so exp never overflows; softmax is shift-invariant.
"""

import numpy as np
from dataclasses import dataclass

import concourse.bass as bass
import concourse.bacc as bacc
import concourse.mybir as mybir
from concourse.tile import TileContext
from concourse.bass_utils import run_bass_kernel_spmd

F32 = mybir.dt.float32
BF16 = mybir.dt.bfloat16
I16 = mybir.dt.int16
AF = mybir.ActivationFunctionType
ALU = mybir.AluOpType

P = 128
IN_CH = 128
OUT_CH = 32
HEADS = 4
RB = 256          # table row width in bf16 elems (512 bytes)
RF = RB // 2      # row width in f32 elems (128)
MW = IN_CH + 8    # stage-A matmul width: 128 xm + 4 s_src + 4 s_tgt
NR = 192          # num-buffer row width in f32 (128 num + 4 den + pad)
EPS = 1.0 + 1e-06
SLOPE = 0.01
NEG = -7000.0     # pad s_src; lrelu -> -70, exp -> 4e-31 (HW-safe, no clamp)
GCALL = 4         # dma_gather columns per call (4*128 = 512 idxs; 1024 is
                  # the HW max but runs at the edge -- 512 adds margin)
GC = 32           # gather super-chunk width in columns (SBUF G tile)
TREE_MIN = 12     # use bf16 tree-reduce for Kt >= this, tensor_reduce below


@dataclass
class Cfg:
    N: int = 50000
    C: int = 8
    SPLIT: int = 32000    # source segment split (int16 range for dma_gather)

    @property
    def NPC(self):
        return self.N // self.C

    @property
    def T(self):
        return -(-self.NPC // P)

    @property
    def RPC(self):
        return self.T * P

    @property
    def NTAB(self):       # rows: 0 dummy | 1..N nodes | N+1 dummy
        return self.N + 2

    @property
    def NBROWS(self):     # num-buffer rows: NPC real + trash + spare
        return self.NPC + 2


def _wrap16(seq: np.ndarray) -> np.ndarray:
    """int sequence [n] (n % 16 == 0) -> dma_gather idx layout [128, n/16]."""
    w = seq.reshape(-1, 16).T.astype(np.int16)
    return np.ascontiguousarray(np.tile(w, (8, 1)))


def _prep_structure(cfg: Cfg, i, j, v, seg: int):
    """Build per-core blobs for one (edge set, source segment) structure.

    seg 0: sources < SPLIT (table view rows [0, S+1), idx = src+1, pad 0)
    seg 1: sources >= SPLIT (view rows [S+1, N+2), idx = src-S, pad N-S)
    Returns (K, offs, SK, per_core list of dicts).
    """
    NPC, T, RPC, N, S = cfg.NPC, cfg.T, cfg.RPC, cfg.N, cfg.SPLIT
    if seg == 0:
        m_seg = j < S
    else:
        m_seg = j >= S
    i, j, v = i[m_seg], j[m_seg], v[m_seg]
    core = i // NPC

    percore = []
    for c in range(cfg.C):
        m = core == c
        il = (i[m] - c * NPC).astype(np.int64)
        jl = j[m].astype(np.int64)
        vl = v[m]
        deg = np.bincount(il, minlength=NPC)
        order = np.argsort(-deg, kind="stable")
        rank = np.empty(NPC, np.int64)
        rank[order] = np.arange(NPC)
        percore.append((il, jl, vl, deg, order, rank))

    K = np.zeros(T, np.int64)
    for c in range(cfg.C):
        ds = np.zeros(RPC, np.int64)
        ds[:NPC] = percore[c][3][percore[c][4]]
        K = np.maximum(K, ds.reshape(T, P).max(axis=1))
    K = np.maximum(K, 1)
    offs = np.concatenate([[0], np.cumsum(K)]).astype(np.int64)
    SK = int(offs[-1])

    # view-local pad index (dummy table row)
    pad_idx = 0 if seg == 0 else N - S
    out = []
    for c in range(cfg.C):
        il, jl, vl, deg, order, rank = percore[c]
        # gather index grid [SK cols, 128 rows]; flat order = col*128 + p
        gidx = np.full((SK, P), pad_idx, np.int64)
        val_blob = np.ones((P, SK), np.float32)
        pos = rank[il]
        o = np.argsort(pos, kind="stable")
        pos_s = pos[o]
        slot = np.arange(len(pos_s)) - np.searchsorted(pos_s, pos_s)
        t = pos_s // P
        p = pos_s % P
        col = offs[t] + slot
        # view-local source index
        jloc = jl[o] + 1 if seg == 0 else jl[o] - S
        gidx[col, p] = jloc
        val_blob[p, col] = vl[o]

        # scatter: local node id per (tile, partition); pads -> trash row NPC
        pos_all = np.arange(RPC)
        valid = pos_all < NPC
        loc = np.full(RPC, NPC, np.int64)
        loc[valid] = order[pos_all[valid]]

        out.append(dict(
            gidx16=_wrap16(gidx.reshape(-1)),          # [128, SK*8]
            val=val_blob,                              # [P, SK]
            scat16=_wrap16(loc),                       # [128, T*8]
            perm=order,                                # host: for xoP build
        ))
    return K, offs, SK, out


def _chunks(K, offs, width=GC):
    """Group consecutive tiles into gather chunks of <= width columns
    (a single tile wider than `width` gets its own chunk).
    Returns list of (col_start, col_width, [tile indices])."""
    res = []
    cur: list[int] = []
    for t in range(len(K)):
        kt = int(K[t])
        if cur and int(offs[t] + kt - offs[cur[0]]) > width:
            res.append((int(offs[cur[0]]), int(offs[cur[-1] + 1] - offs[cur[0]]), cur))
            cur = []
        cur.append(t)
    res.append((int(offs[cur[0]]), int(offs[cur[-1] + 1] - offs[cur[0]]), cur))
    return res


def _build_nc(cfg: Cfg, meta):
    """meta: dict (set,seg) -> (K, offs, SK)."""
    sets = ("low", "up")
    N, NPC, T, NTAB, S = cfg.N, cfg.NPC, cfg.T, cfg.NTAB, cfg.SPLIT
    RPC, NBR = cfg.RPC, cfg.NBROWS
    NBF = NBR * NR          # flat f32 size of one num buffer
    assert NBF % P == 0
    NBW = NBF // P          # zero-fill row width


    nc = bacc.Bacc("TRN2", target_bir_lowering=False, debug=False,
                   num_swdge_queues=4)

    xT = nc.dram_tensor("xT", [P, N], F32, kind="ExternalInput")
    xoT = nc.dram_tensor("xoT", [P, NPC], F32, kind="ExternalInput")
    Wsk = nc.dram_tensor("W_skip", [IN_CH, HEADS * OUT_CH], F32, kind="ExternalInput")
    W = {s: nc.dram_tensor(f"W_{s}", [HEADS, IN_CH, OUT_CH], F32, kind="ExternalInput")
         for s in sets}
    A = {s: nc.dram_tensor(f"ab_{s}", [P, 2 * HEADS], F32, kind="ExternalInput")
         for s in sets}

    gidx_d, val_d, scat_d, xop_d = {}, {}, {}, {}
    for key, (K, offs, SK) in meta.items():
        s, g = key
        gidx_d[key] = nc.dram_tensor(f"gidx_{s}{g}", [P, SK * 8], I16,
                                     kind="ExternalInput")
        val_d[key] = nc.dram_tensor(f"val_{s}{g}", [P, SK], F32,
                                    kind="ExternalInput")
        scat_d[key] = nc.dram_tensor(f"scat_{s}{g}", [P, T * 8], I16,
                                     kind="ExternalInput")
        xop_d[key] = nc.dram_tensor(f"xop_{s}{g}", [P, RPC], F32,
                                    kind="ExternalInput")

    tab = {s: nc.dram_tensor(f"tab_{s}", [NTAB * RB], BF16) for s in sets}
    numb = {(s, g): nc.dram_tensor(f"numb_{s}{g}", [NBF], F32)
            for s in sets for g in range(2)}
    out_d = nc.dram_tensor("out", [NPC, IN_CH], F32, kind="ExternalOutput")

    n_atiles = -(-N // P)

    with TileContext(nc) as tc:
        with (
            tc.tile_pool(name="const", bufs=1) as cpool,
            tc.tile_pool(name="setup_ps", bufs=1, space="PSUM") as spsum,
            tc.tile_pool(name="stagea", bufs=3) as apool,
            tc.tile_pool(name="stagea_ps", bufs=3, space="PSUM") as apsum,
            tc.tile_pool(name="blob", bufs=1) as bpool,
            tc.tile_pool(name="a2_ps", bufs=1, space="PSUM") as a2psum,
            tc.tile_pool(name="gather", bufs=2) as gpool,
            tc.tile_pool(name="small", bufs=3) as upool,
            tc.tile_pool(name="fin", bufs=3) as fpool,
            tc.tile_pool(name="fin_ps", bufs=2, space="PSUM") as fpsum,
        ):
            from concourse.masks import make_identity

            ident = cpool.tile([P, P], F32)
            make_identity(nc, ident[:])
            wsk_sb = cpool.tile([IN_CH, HEADS * OUT_CH], F32)
            nc.sync.dma_start(out=wsk_sb[:], in_=Wsk[:, :])

            # ---- zero the num buffers (overlaps with stage A) ----
            # written through the same [rows, NR] view the scatter-adds use,
            # so the WAR/RAW ordering is tracked on identical APs
            zt = cpool.tile([P, NR], F32)
            nc.vector.memset(zt[:], 0.0)
            for key in numb:
                nv = numb[key][:].rearrange("(n e) -> n e", e=NR)
                for q in range(0, NBR, P):
                    n = min(P, NBR - q)
                    nc.sync.dma_start(out=nv[q:q + n, :], in_=zt[:n, :])

            # ---- M matrices: [128 in_ch, 136]: cols 0-127 W heads,
            #      128-131 src score cols, 132-135 tgt score cols
            M = {}
            for s in sets:
                m_sb = cpool.tile([IN_CH, MW], F32, tag=f"M_{s}")
                nc.vector.memset(m_sb[:], 0.0)
                nc.sync.dma_start(
                    out=m_sb[:, 0:IN_CH].rearrange("p (h f) -> p h f", h=HEADS),
                    in_=W[s][:, :, :].rearrange("h p f -> p h f"),
                )
                wt_ps = spsum.tile([P, P], F32, tag="wt_ps")
                nc.tensor.transpose(out=wt_ps[:], in_=m_sb[:, 0:IN_CH],
                                    identity=ident[:])
                wt_sb = upool.tile([P, P], F32, tag="wt_sb")
                nc.vector.tensor_copy(out=wt_sb[:], in_=wt_ps[:])
                a_bd = upool.tile([P, 2 * HEADS], F32, tag="a_bd")
                nc.sync.dma_start(out=a_bd[:], in_=A[s][:, :])
                col_ps = spsum.tile([P, 8], F32, tag="col_ps")
                nc.tensor.matmul(out=col_ps[:], lhsT=wt_sb[:], rhs=a_bd[:],
                                 start=True, stop=True)
                nc.vector.tensor_copy(out=m_sb[:, IN_CH:IN_CH + 8], in_=col_ps[:])
                M[s] = m_sb

            # ---- stage A: build tables ----
            tabv = {s: tab[s][:].rearrange("(n e) -> n e", e=RB) for s in sets}
            tabf = {s: tab[s][:].bitcast(F32).rearrange("(n e) -> n e", e=RF)
                    for s in sets}
            for ti in range(n_atiles):
                st = ti * P
                n = min(P, N - st)
                xt = apool.tile([P, n], F32, tag="xt")
                nc.sync.dma_start(out=xt[:], in_=xT[:, st:st + n])
                for s in sets:
                    ps = apsum.tile([P, MW], F32, tag="a_ps")
                    nc.tensor.matmul(out=ps[:n, :], lhsT=xt[:], rhs=M[s][:],
                                     start=True, stop=True)
                    sb = apool.tile([P, RB], BF16, tag=f"a_sb_{s}")
                    nc.scalar.activation(
                        out=sb[:n, 0:IN_CH], in_=ps[:n, 0:IN_CH], func=AF.Copy,
                    )
                    nc.vector.tensor_copy(
                        out=sb[:].bitcast(F32)[:n, RF // 2:RF // 2 + 4],
                        in_=ps[:n, IN_CH:IN_CH + 4],
                    )
                    nc.sync.dma_start(out=tabv[s][1 + st:1 + st + n, :],
                                      in_=sb[:n, :])
            # dummy rows 0 and N+1: xm=0, s_src=NEG
            dmy = cpool.tile([1, RF], F32)
            nc.vector.memset(dmy[:], 0.0)
            nc.vector.memset(dmy[:, RF // 2:RF // 2 + 4], NEG)
            for s in sets:
                nc.sync.dma_start(out=tabf[s][0:1, :], in_=dmy[:])
                nc.sync.dma_start(out=tabf[s][N + 1:N + 2, :], in_=dmy[:])

            numv = {k: numb[k][:].rearrange("(n e) -> n e", e=NR)
                    for k in numb}

            # ---- stage B: 4 structures ----
            _gq = [0]     # gather queue toggle
            for s in sets:
                for g in range(2):
                    key = (s, g)
                    K, offs, SK = meta[key]
                    gi16 = bpool.tile([P, SK * 8], I16, tag="gi16")
                    nc.sync.dma_start(out=gi16[:], in_=gidx_d[key][:, :])
                    vb = bpool.tile([P, SK], F32, tag="vb")
                    nc.sync.dma_start(out=vb[:], in_=val_d[key][:, :])
                    sc16 = bpool.tile([P, T * 8], I16, tag="sc16")
                    nc.sync.dma_start(out=sc16[:], in_=scat_d[key][:, :])
                    xoP = bpool.tile([P, RPC], F32, tag="xoP")
                    nc.sync.dma_start(out=xoP[:], in_=xop_d[key][:, :])

                    # stage A2: s_tgt for own (permuted) targets
                    stps = a2psum.tile([P, T * HEADS], F32, tag="stps")
                    for t in range(T):
                        nc.tensor.matmul(
                            out=stps[:, t * HEADS:(t + 1) * HEADS],
                            lhsT=xoP[:, t * P:(t + 1) * P],
                            rhs=M[s][:, IN_CH + 4:IN_CH + 8],
                            start=True, stop=True)
                    stP = bpool.tile([P, T * HEADS], F32, tag="stP")
                    nc.vector.tensor_copy(out=stP[:], in_=stps[:])
                    stPv = stP[:].rearrange("p (t h) -> p t h", h=HEADS)

                    na = bpool.tile([P, T * NR], F32, tag="na")
                    nav = na[:].rearrange("p (t q) -> p t q", q=NR)

                    # table view for this segment (int16 index coverage)
                    view = (tabv[s][0:S + 1, :] if g == 0
                            else tabv[s][S + 1:N + 2, :])

                    for (c0, cw, tiles) in _chunks(K, offs):
                        G = gpool.tile([P, cw * RB], BF16, tag="G")
                        Gg = G[:].rearrange("p (c e) -> p c e", e=RB)
                        for q0 in range(0, cw, GCALL):
                            k = min(GCALL, cw - q0)
                            # alternate SWDGE queues: each queue's desc-gen
                            # runs on its own Q7 core pair
                            nc.gpsimd.dma_gather(
                                out_ap=Gg[:, q0:q0 + k, :],
                                in_ap=view,
                                idxs_ap=gi16[:, (c0 + q0) * 8:(c0 + q0 + k) * 8],
                                num_idxs=k * P,
                                num_idxs_reg=k * P,
                                elem_size=RB,
                                queue_num=_gq[0],
                            )
                            _gq[0] = (_gq[0] + 1) % 4
                        Gf = G[:].bitcast(F32).rearrange("p (c q) -> p c q", q=RF)
                        G4 = G[:].rearrange("p (c h f) -> p c h f",
                                            h=RB // OUT_CH, f=OUT_CH)[:, :, 0:HEADS, :]
                        Gr = G[:].rearrange("p (c h f) -> p h f c",
                                            h=RB // OUT_CH, f=OUT_CH)[:, 0:HEADS, :, :]
                        Gxm = Gg[:, :, 0:IN_CH]
                        for t in tiles:
                            Kt = int(K[t])
                            lc = int(offs[t]) - c0
                            u4 = upool.tile([P, Kt * HEADS], F32, tag="u4")
                            u4v = u4[:].rearrange("p (k h) -> p k h", h=HEADS)
                            # (s_src + s_tgt) * val
                            nc.vector.tensor_add(
                                out=u4v,
                                in0=Gf[:, lc:lc + Kt, RF // 2:RF // 2 + 4],
                                in1=stPv[:, t:t + 1, :]
                                .to_broadcast([P, Kt, HEADS]),
                            )
                            nc.vector.tensor_mul(
                                out=u4v, in0=u4v,
                                in1=vb[:, int(offs[t]):int(offs[t]) + Kt]
                                .rearrange("p (k o) -> p k o", o=1)
                                .to_broadcast([P, Kt, HEADS]),
                            )
                            # leaky relu (val >= 0); pads underflow exp
                            nc.vector.scalar_tensor_tensor(
                                out=u4[:], in0=u4[:], scalar=SLOPE, in1=u4[:],
                                op0=ALU.mult, op1=ALU.max,
                            )
                            nc.scalar.activation(out=u4[:], in_=u4[:], func=AF.Exp)
                            # den per head
                            nc.vector.tensor_reduce(
                                out=nav[:, t:t + 1, IN_CH:IN_CH + HEADS],
                                in_=u4[:].rearrange("p (k h) -> p h k", h=HEADS),
                                axis=mybir.AxisListType.X, op=ALU.add,
                            )
                            # scale gathered xm by exp-score
                            nc.vector.tensor_mul(
                                out=G4[:, lc:lc + Kt, :, :],
                                in0=G4[:, lc:lc + Kt, :, :],
                                in1=u4[:].rearrange(
                                    "p (k h o) -> p k h o", h=HEADS, o=1)
                                .to_broadcast([P, Kt, HEADS, OUT_CH]),
                            )
                            # reduce over slots -> num
                            if Kt >= TREE_MIN:
                                m_, base = Kt, lc
                                while m_ > 1:
                                    h_ = m_ // 2
                                    so = m_ - h_
                                    nc.vector.tensor_add(
                                        out=Gxm[:, base:base + h_, :],
                                        in0=Gxm[:, base:base + h_, :],
                                        in1=Gxm[:, base + so:base + m_, :],
                                    )
                                    m_ = so
                                nc.vector.tensor_copy(
                                    out=nav[:, t:t + 1, 0:IN_CH],
                                    in_=Gxm[:, base:base + 1, :],
                                )
                            else:
                                nc.vector.tensor_reduce(
                                    out=nav[:, t:t + 1, 0:IN_CH],
                                    in_=Gr[:, :, :, lc:lc + Kt],
                                    axis=mybir.AxisListType.X, op=ALU.add,
                                )
                    # batched scatter-add into the node-ordered num buffer
                    for q0 in range(0, T, GCALL):
                        k = min(GCALL, T - q0)
                        nc.gpsimd.dma_scatter_add(
                            out_ap=numv[key],
                            in_ap=na[:, q0 * NR:(q0 + k) * NR]
                            .rearrange("p (c e) -> p c e", e=NR),
                            idxs_ap=sc16[:, q0 * 8:(q0 + k) * 8],
                            num_idxs=k * P,
                            num_idxs_reg=k * P,
                            elem_size=NR,
                            queue_num=_gq[0],
                        )
                        _gq[0] = (_gq[0] + 1) % 4

            # ---- final pass ----
            n_ftiles = -(-NPC // P)
            for t in range(n_ftiles):
                st = t * P
                n = min(P, NPC - st)
                xo = fpool.tile([P, n], F32, tag="xo")
                nc.sync.dma_start(out=xo[:], in_=xoT[:, st:st + n])
                ps = fpsum.tile([P, IN_CH], F32, tag="f_ps")
                nc.tensor.matmul(out=ps[:n, :], lhsT=xo[:], rhs=wsk_sb[:],
                                 start=True, stop=True)
                # each edge set normalizes by its OWN denominator
                accs = []
                for s in sets:
                    nb = fpool.tile([P, NR], F32, tag=f"nb_{s}")
                    nc.sync.dma_start(out=nb[:n, :], in_=numv[(s, 0)][st:st + n, :])
                    nb1 = fpool.tile([P, NR], F32, tag=f"nb1_{s}")
                    nc.sync.dma_start(out=nb1[:n, :],
                                      in_=numv[(s, 1)][st:st + n, :])
                    nc.vector.tensor_add(out=nb[:n, :], in0=nb[:n, :],
                                         in1=nb1[:n, :])
                    rec = upool.tile([P, HEADS], F32, tag=f"rec_{s}")
                    nc.vector.tensor_scalar_max(
                        out=rec[:n], in0=nb[:n, IN_CH:IN_CH + HEADS],
                        scalar1=1e-16)
                    nc.vector.reciprocal(out=rec[:n], in_=rec[:n])
                    acc = fpool.tile([P, IN_CH], F32, tag=f"acc_{s}")
                    nc.vector.tensor_mul(
                        out=acc[:n].rearrange("p (h f) -> p h f", h=HEADS),
                        in0=nb[:n, 0:IN_CH].rearrange("p (h f) -> p h f", h=HEADS),
                        in1=rec[:n].rearrange("p (h o) -> p h o", o=1)
                        .to_broadcast([n, HEADS, OUT_CH]),
                    )
                    accs.append(acc)
                acc = accs[0]
                nc.vector.tensor_add(out=acc[:n, :], in0=acc[:n, :],
                                     in1=accs[1][:n, :])
                # acc += EPS * skip; relu
                nc.vector.scalar_tensor_tensor(
                    out=acc[:n, :], in0=ps[:n, :], scalar=EPS, in1=acc[:n, :],
                    op0=ALU.mult, op1=ALU.add,
                )
                nc.scalar.activation(out=acc[:n, :], in_=acc[:n, :], func=AF.Relu)
                nc.sync.dma_start(out=out_d[st:st + n, :], in_=acc[:n, :])

    nc.compile()
    return nc


def _prep_all(cfg: Cfg, inputs):
    x = np.asarray(inputs["x"], dtype=np.float32)
    xTf = np.ascontiguousarray(x.T)

    meta, blobs = {}, {}
    for s, ikey, vkey in (
        ("low", "lower_indices", "lower_values"),
        ("up", "upper_indices", "upper_values"),
    ):
        idx = np.asarray(inputs[ikey])
        i = np.asarray(idx[0], dtype=np.int64)
        j = np.asarray(idx[1], dtype=np.int64)
        v = np.asarray(inputs[vkey], dtype=np.float32)
        for g in range(2):
            K, offs, SK, pc = _prep_structure(cfg, i, j, v, g)
            meta[(s, g)] = (K, offs, SK)
            blobs[(s, g)] = pc

    nc = _build_nc(cfg, meta)

    def ablob(a):
        a = np.asarray(a, dtype=np.float32)
        ab = np.zeros((P, 2 * HEADS), np.float32)
        for h in range(HEADS):
            ab[h * OUT_CH:(h + 1) * OUT_CH, h] = a[h, :OUT_CH]
            ab[h * OUT_CH:(h + 1) * OUT_CH, HEADS + h] = a[h, OUT_CH:]
        return ab

    NPC, RPC = cfg.NPC, cfg.RPC
    in_maps = []
    for c in range(cfg.C):
        m = {
            "xT": xTf,
            "xoT": np.ascontiguousarray(xTf[:, c * NPC:(c + 1) * NPC]),
            "W_skip": np.asarray(inputs["W_skip"], dtype=np.float32),
            "W_low": np.asarray(inputs["W_low"], dtype=np.float32),
            "ab_low": ablob(inputs["a_low"]),
            "W_up": np.asarray(inputs["W_up"], dtype=np.float32),
            "ab_up": ablob(inputs["a_up"]),
        }
        for key, pc in blobs.items():
            s, g = key
            b = pc[c]
            xop = np.zeros((P, RPC), np.float32)
            xop[:, :NPC] = xTf[:, b["perm"] + c * NPC]
            m[f"gidx_{s}{g}"] = b["gidx16"]
            m[f"val_{s}{g}"] = b["val"]
            m[f"scat_{s}{g}"] = b["scat16"]
            m[f"xop_{s}{g}"] = xop
        in_maps.append(m)
    return nc, in_maps


def _execute(inputs, trace=False, trace_cores=None, cfg=None):
    if cfg is None:
        cfg = Cfg(N=int(np.asarray(inputs["x"]).shape[0]), C=8)
    nc, in_maps = _prep_all(cfg, inputs)
    res = run_bass_kernel_spmd(
        nc, in_maps, core_ids=list(range(cfg.C)),
        trace=trace, trace_cores=trace_cores,
    )
    out = np.concatenate([res.results[c]["out"] for c in range(cfg.C)], axis=0)
    return out.astype(np.float32), res


def kernel(**inputs) -> np.ndarray:
    return _execute(inputs)[0]
